# revision 2
# baseline (speedup 1.0000x reference)
"""Trainium2 Bass kernel for BlockChunkedActivityRoutedNet.

Reference computation (B=4096, IN_F=4096, 8 chunks of 512, top-2 by mean|x|,
chunk-expert Linears 512->512, concat -> final Linear 1024->4096):

    xr = x.reshape(B, 8, 512)
    activities = mean(|xr|, axis=(0, 2))            # over the WHOLE batch
    i0, i1 = top2(activities)                        # descending
    h = concat(xr[:, i0] @ Wc[i0] + bc[i0], xr[:, i1] @ Wc[i1] + bc[i1])
    out = h @ W_final + b_final

Distribution: data-parallel over the batch across 8 NeuronCores (512 rows
each). Per-chunk |x| partial sums are AllReduced (tiny [1,8] collective) so
every core computes the identical top-2 routing. Chunk selection is done
entirely with data-driven indirect DMA gathers (no sequencer registers --
reg_load faults on this runtime). Matmuls run in bf16 (1 cyc/row on PE)
with fp32 PSUM accumulation.

Host-side prep inside kernel(): casts weights/activations to bf16 and
pre-transposes each core's x shard to xT [feat, batch] (a layout change,
not computation) so the device needs no PE transposes and reads the
contraction dim on partitions directly.

Per core:
  activities: stream xT [4096,512] bf16 in 4 chunks, |.|-reduce over batch
              (DVE on half the feature tiles, ScalarE Abs+accum on the other
              half), partition-reduce via ones-matmul -> [1,32] -> [1,8],
              AllReduce, top2 via vector.max/max_index.
  routing:    broadcast top-2 indices to 128 partitions via K=1 matmul,
              build per-partition row offsets with iota + DVE math; chunk
              biases selected on-chip (PE-transpose of b_chunks + one-hot).
  gather:     8+8 indirect-DMA row gathers: xT[sel] and W_chunks[sel]
              k-tiles (row index = sel*512 + kt*128 + p for both tables).
  L1:         hT[s][d] = matmul(lhsT=Wk k-tiles, rhs=xT_sel k-tiles) over 4
              k-tiles; bias added during PSUM->SBUF eviction -> bf16.
  L2:         out[b, o] = matmul(lhsT=hT slice, rhs=W_final k-tiles) over 8
              k-tiles; b_final added by DVE during PSUM eviction; DMA out.
"""

import os
import numpy as np
import ml_dtypes

import concourse.bass as bass
import concourse.bacc as bacc
import concourse.mybir as mybir
from concourse.tile import TileContext
from concourse.bass_utils import run_bass_kernel_spmd
from concourse.masks import make_identity

dt = mybir.dt
P = 128

NUM_CHUNKS = 8
TOP_K = 2
IN_F = 4096
HID_F = 4096
OUT_F = 4096
B = 4096
CIN = IN_F // NUM_CHUNKS      # 512
COUT = HID_F // NUM_CHUNKS    # 512
N_CORES = 8
BS = B // N_CORES             # 512 rows per core

BT = BS // P                  # 4 batch tiles per core
KT = CIN // P                 # 4 k-tiles per selected chunk
DT_ = COUT // P               # 4 d-tiles per selected chunk
KF = TOP_K * DT_              # 8 k-tiles for the final matmul
OT = OUT_F // 512             # 8 output column tiles of 512
FT = IN_F // P                # 32 feature tiles of xT
FG = 8                        # xT activity-load groups
WF_BUFS = 64                  # all of W_final bf16 resident (64 x 128KB)

_cache = {}


def _build():
    nc = bacc.Bacc(num_devices=N_CORES, name="chunk_routed_net",
                   num_swdge_queues=4)

    xT = nc.dram_tensor("xT_shard", [IN_F, BS], dt.bfloat16,
                        kind="ExternalInput")
    Wc = nc.dram_tensor("W_chunks", [NUM_CHUNKS, CIN, COUT], dt.bfloat16,
                        kind="ExternalInput")
    bc_t = nc.dram_tensor("b_chunks", [NUM_CHUNKS, COUT], dt.float32,
                          kind="ExternalInput")
    Wf = nc.dram_tensor("W_final", [COUT * TOP_K, OUT_F], dt.bfloat16,
                        kind="ExternalInput")
    bf = nc.dram_tensor("b_final", [1, OUT_F], dt.float32, kind="ExternalInput")
    out = nc.dram_tensor("out_shard", [BS, OUT_F], dt.float32,
                         kind="ExternalOutput")

    Wc_rows = Wc[:].rearrange("a b c -> (a b) c")          # [4096, 512] bf16

    with TileContext(nc) as tc:
        with tc.tile_pool(name="consts", bufs=1) as consts, \
             tc.tile_pool(name="route", bufs=1) as route, \
             tc.tile_pool(name="xl", bufs=1) as xl_pool, \
             tc.tile_pool(name="gath", bufs=1) as gath, \
             tc.tile_pool(name="hts", bufs=1) as hts, \
             tc.tile_pool(name="bfinp", bufs=1) as bfinp, \
             tc.tile_pool(name="wfs", bufs=WF_BUFS) as wfs, \
             tc.tile_pool(name="outs", bufs=4) as outs, \
             tc.tile_pool(name="dram", bufs=1, space="DRAM") as dram:

            # -------- early dummy collective: pre-pay barrier/ring setup ----
            dummy_in = dram.tile([1, NUM_CHUNKS], dt.float32)
            dummy_out = dram.tile([1, NUM_CHUNKS], dt.float32)
            nc.gpsimd.collective_compute(
                "AllReduce", mybir.AluOpType.add,
                replica_groups=[list(range(N_CORES))],
                ins=[dummy_in.opt()], outs=[dummy_out.opt()])

            # ---------------- constants ----------------
            ones_col = consts.tile([P, 1], dt.float32)     # partition reduce
            nc.vector.memset(ones_col[:], 1.0)
            ones_k1 = consts.tile([1, P], dt.float32)      # K=1 bcast matmul
            nc.vector.memset(ones_k1[:], 1.0)
            ones_k1h = consts.tile([1, P], dt.bfloat16)    # K=1 bf16 bcast
            nc.vector.memset(ones_k1h[:], 1.0)
            ident = consts.tile([P, P], dt.float32)
            make_identity(nc, ident)
            # C_W[p, j] = p + 128*j       (row offset within a chunk)
            C_W = consts.tile([P, KT], dt.int32)
            nc.gpsimd.iota(C_W[:], pattern=[[P, KT]], base=0, channel_multiplier=1)
            C_Wf = consts.tile([P, KT], dt.float32)
            nc.vector.tensor_copy(C_Wf[:], C_W[:])
            # C8[p, c] = c                (chunk-id iota along free dim)
            C8 = consts.tile([P, NUM_CHUNKS], dt.int32)
            nc.gpsimd.iota(C8[:], pattern=[[1, NUM_CHUNKS]], base=0,
                           channel_multiplier=0)
            C8f = consts.tile([P, NUM_CHUNKS], dt.float32)
            nc.vector.tensor_copy(C8f[:], C8[:])

            with tc.tile_pool(name="ps_early", bufs=1, space="PSUM") as ps_early:
                # ------------ activities from xT (chunk-aligned loads) -----
                # chunk c rows of view [1024, 2048] = xT feats 4r..4r+3; rows
                # c*128..c*128+127 are exactly chunk c -> 4KB runs/partition.
                xT_w = xT[:].rearrange("(r q) b -> r (q b)", q=4)  # [1024,2048]
                actcol = route.tile([P, NUM_CHUNKS], dt.float32)
                scr = route.tile([P, 4 * BS], dt.bfloat16)  # ACT throwaway
                xls = []
                H = 2 * BS
                for c in range(NUM_CHUNKS):
                    xlt = xl_pool.tile([P, 4 * BS], dt.bfloat16, tag=f"xl{c}",
                                       name=f"xl{c}")
                    nc.sync.dma_start(xlt[:, 0:H],
                                      xT_w[c * P:(c + 1) * P, 0:H])
                    nc.sync.dma_start(xlt[:, H:2 * H],
                                      xT_w[c * P:(c + 1) * P, H:2 * H])
                    xls.append(xlt)
                for c in range(NUM_CHUNKS):
                    if c % 2 == 0:
                        nc.vector.tensor_reduce(
                            actcol[:, c:c + 1], xls[c][:],
                            axis=mybir.AxisListType.X, op=mybir.AluOpType.add,
                            apply_absolute_value=True)
                    else:
                        nc.scalar.activation(
                            scr[:], xls[c][:],
                            mybir.ActivationFunctionType.Abs,
                            accum_out=actcol[:, c:c + 1])
                act_ps = ps_early.tile([1, NUM_CHUNKS], dt.float32, tag="psa")
                nc.tensor.matmul(act_ps[:], ones_col[:], actcol[:],
                                 start=True, stop=True)
                act_l = route.tile([1, NUM_CHUNKS], dt.float32)
                nc.scalar.copy(act_l[:], act_ps[:])

                # ------------ AllReduce ------------
                cc_in = dram.tile([1, NUM_CHUNKS], dt.float32)
                cc_out = dram.tile([1, NUM_CHUNKS], dt.float32)
                nc.sync.dma_start(cc_in[:], act_l[:])
                nc.gpsimd.collective_compute(
                    "AllReduce", mybir.AluOpType.add,
                    replica_groups=[list(range(N_CORES))],
                    ins=[cc_in.opt()], outs=[cc_out.opt()])

                # ---- work that fills the AllReduce wait ----
                # W_final prefetch (sync queue; past cc_in trigger by now)
                wf_tiles = {}
                for o in range(OT):
                    osl = slice(o * 512, (o + 1) * 512)
                    for kf in range(KF):
                        w = wfs.tile([P, 512], dt.bfloat16, tag="wf",
                                     name=f"wf{o}_{kf}")
                        nc.sync.dma_start(
                            w[:], Wf[kf * P:(kf + 1) * P, osl])
                        wf_tiles[(o, kf)] = w
                # b_final broadcast [128, 4096]
                bfin = bfinp.tile([1, OUT_F], dt.float32)
                nc.scalar.dma_start(bfin[:], bf[:])
                bfin_h = bfinp.tile([1, OUT_F], dt.bfloat16)
                nc.vector.tensor_copy(bfin_h[:], bfin[:])
                bfin_bc = bfinp.tile([P, OUT_F], dt.float32)
                for o in range(OT):
                    sl = slice(o * 512, (o + 1) * 512)
                    ps_b = ps_early.tile([P, 512], dt.float32, tag="psb")
                    nc.tensor.matmul(ps_b[:], ones_k1h[:], bfin_h[:, sl],
                                     start=True, stop=True)
                    nc.vector.tensor_copy(bfin_bc[:, sl], ps_b[:])
                # b_chunks transpose
                b_sb = route.tile([NUM_CHUNKS, COUT], dt.float32)
                nc.scalar.dma_start(b_sb[:], bc_t[:])
                bT = route.tile([P, DT_ * NUM_CHUNKS], dt.float32)
                for d in range(DT_):
                    ps_t = ps_early.tile([P, NUM_CHUNKS], dt.float32, tag="pst")
                    nc.tensor.transpose(
                        ps_t[:], b_sb[:, d * P:(d + 1) * P],
                        ident[0:NUM_CHUNKS, 0:NUM_CHUNKS])
                    nc.scalar.copy(bT[:, d * NUM_CHUNKS:(d + 1) * NUM_CHUNKS],
                                   ps_t[:])

                act_g = route.tile([1, NUM_CHUNKS], dt.float32)
                nc.gpsimd.dma_start(act_g[:], cc_out[:])

                # ------------ top-2 ------------
                maxv = route.tile([1, NUM_CHUNKS], dt.float32)
                maxi = route.tile([1, NUM_CHUNKS], dt.uint32)
                nc.vector.max(maxv[:], act_g[:])
                nc.vector.max_index(maxi[:], maxv[:], act_g[:])
                maxi_f = route.tile([1, NUM_CHUNKS], dt.float32)
                nc.vector.tensor_copy(maxi_f[:], maxi[:])

                # bcast[p, j] = idx[j] on every partition (K=1 matmul)
                bc_ps = ps_early.tile([P, NUM_CHUNKS], dt.float32, tag="psc")
                nc.tensor.matmul(bc_ps[:], ones_k1[:], maxi_f[:],
                                 start=True, stop=True)
                bcast = route.tile([P, NUM_CHUNKS], dt.float32)
                nc.vector.tensor_copy(bcast[:], bc_ps[:])

            # gather offsets: offW[p, s*4+kt] = sel_s*512 + kt*128 + p
            bc512 = route.tile([P, TOP_K], dt.float32)
            nc.vector.tensor_scalar_mul(bc512[:], bcast[:, 0:TOP_K], 512.0)
            offW_f = route.tile([P, TOP_K * KT], dt.float32)
            for s in range(TOP_K):
                nc.vector.tensor_scalar(
                    offW_f[:, s * KT:(s + 1) * KT], C_Wf[:],
                    bc512[:, s:s + 1], scalar2=None, op0=mybir.AluOpType.add)
            offW = route.tile([P, TOP_K * KT], dt.int32)
            nc.vector.tensor_copy(offW[:], offW_f[:])

            # ------------ gathers: xT[sel] then W[sel], chunk 0 first ------
            xTg = [[gath.tile([P, BS], dt.bfloat16, tag=f"xg{s}_{kt}",
                              name=f"xg{s}_{kt}")
                    for kt in range(KT)] for s in range(TOP_K)]
            Wk = [[gath.tile([P, COUT], dt.bfloat16, tag=f"wk{s}_{kt}",
                             name=f"wk{s}_{kt}")
                   for kt in range(KT)] for s in range(TOP_K)]
            for s in range(TOP_K):
                for kt in range(KT):
                    nc.gpsimd.indirect_dma_start(
                        out=xTg[s][kt][:], out_offset=None,
                        in_=xT[:],
                        in_offset=bass.IndirectOffsetOnAxis(
                            ap=offW[:, s * KT + kt:s * KT + kt + 1], axis=0))
                    nc.gpsimd.indirect_dma_start(
                        out=Wk[s][kt][:], out_offset=None,
                        in_=Wc_rows,
                        in_offset=bass.IndirectOffsetOnAxis(
                            ap=offW[:, s * KT + kt:s * KT + kt + 1], axis=0))

            # chunk-bias select: bias[s][d][p] = bT[p, d*8 + sel_s]
            onehot = route.tile([P, TOP_K * NUM_CHUNKS], dt.float32)
            for s in range(TOP_K):
                nc.vector.tensor_scalar(
                    onehot[:, s * NUM_CHUNKS:(s + 1) * NUM_CHUNKS], C8f[:],
                    bcast[:, s:s + 1], scalar2=None,
                    op0=mybir.AluOpType.is_equal)
            bsel = [[route.tile([P, 1], dt.float32, tag=f"bs{s}_{d}",
                                name=f"bs{s}_{d}")
                     for d in range(DT_)] for s in range(TOP_K)]
            btmp = route.tile([P, NUM_CHUNKS], dt.float32)
            for s in range(TOP_K):
                for d in range(DT_):
                    nc.vector.tensor_tensor(
                        out=btmp[:], in0=bT[:, d * NUM_CHUNKS:(d + 1) * NUM_CHUNKS],
                        in1=onehot[:, s * NUM_CHUNKS:(s + 1) * NUM_CHUNKS],
                        op=mybir.AluOpType.mult)
                    nc.vector.tensor_reduce(
                        bsel[s][d][:], btmp[:], axis=mybir.AxisListType.X,
                        op=mybir.AluOpType.add)

            with tc.tile_pool(name="ps_h", bufs=2, space="PSUM") as ps_h, \
                 tc.tile_pool(name="ps_o", bufs=6, space="PSUM") as ps_o:
                # ------------ L1: hT[s][d] = (x_sel @ Wc[sel]).T + b -------
                hT = [[hts.tile([P, BS], dt.bfloat16, tag=f"ht{s}_{d}",
                                name=f"ht{s}_{d}")
                       for d in range(DT_)] for s in range(TOP_K)]

                def l1_chunk(s):
                    for d in range(DT_):
                        ph = ps_h.tile([P, BS], dt.float32, tag="ph",
                                       name=f"ph{s}_{d}")
                        for kt in range(KT):
                            nc.tensor.matmul(
                                ph[:], Wk[s][kt][:, d * P:(d + 1) * P],
                                xTg[s][kt][:],
                                start=(kt == 0), stop=(kt == KT - 1))
                        nc.scalar.activation(
                            hT[s][d][:], ph[:],
                            mybir.ActivationFunctionType.Identity,
                            bias=bsel[s][d][:, 0:1])

                l1_chunk(0)

                # pre-start six psum groups (all of o=0 plus o=1 bt=0,1)
                # on chunk-0 hT while chunk-1 gathers are still in flight
                # (PE executes in program order, so this fills the wait)
                PRE = [(0, 0), (0, 1), (0, 2), (0, 3), (1, 0), (1, 1)]
                pre = {}
                for (po_, bt) in PRE:
                    po = ps_o.tile([P, 512], dt.float32, tag="po",
                                   name=f"po_pre{po_}_{bt}")
                    for kf in range(DT_):
                        nc.tensor.matmul(
                            po[:], hT[0][kf][:, bt * P:(bt + 1) * P],
                            wf_tiles[(po_, kf)][:],
                            start=(kf == 0), stop=False)
                    pre[(po_, bt)] = po

                l1_chunk(1)

                # ------------ L2: out = h @ W_final + b_final --------------
                for o in range(OT):
                    osl = slice(o * 512, (o + 1) * 512)
                    for bt in range(BT):
                        if (o, bt) in pre:
                            po = pre[(o, bt)]
                            kfs = range(DT_, KF)
                        else:
                            po = ps_o.tile([P, 512], dt.float32, tag="po",
                                           name=f"po{o}_{bt}")
                            kfs = range(KF)
                        for kf in kfs:
                            s, d = divmod(kf, DT_)
                            nc.tensor.matmul(
                                po[:], hT[s][d][:, bt * P:(bt + 1) * P],
                                wf_tiles[(o, kf)][:],
                                start=(kf == 0), stop=(kf == KF - 1))
                        ot_sb = outs.tile([P, 512], dt.float32, tag="ot",
                                          name=f"ot{o}_{bt}")
                        nc.vector.tensor_add(ot_sb[:], po[:], bfin_bc[:, osl])
                        nc.sync.dma_start(
                            out[bt * P:(bt + 1) * P, osl], ot_sb[:])
    nc.compile()
    return nc


def kernel(x, W_chunks, b_chunks, W_final, b_final):
    bf16 = ml_dtypes.bfloat16
    x = np.asarray(x, dtype=np.float32).astype(bf16)
    W_chunks = np.asarray(W_chunks, dtype=np.float32).astype(bf16)
    W_final = np.asarray(W_final, dtype=np.float32).astype(bf16)
    b_chunks = np.ascontiguousarray(np.asarray(b_chunks, dtype=np.float32))
    b_final = np.ascontiguousarray(
        np.asarray(b_final, dtype=np.float32).reshape(1, OUT_F))

    if "nc" not in _cache:
        _cache["nc"] = _build()
    nc = _cache["nc"]

    in_maps = [{
        "xT_shard": np.ascontiguousarray(x[c * BS:(c + 1) * BS].T),
        "W_chunks": W_chunks,
        "b_chunks": b_chunks,
        "W_final": W_final,
        "b_final": b_final,
    } for c in range(N_CORES)]

    res = run_bass_kernel_spmd(nc, in_maps, core_ids=list(range(N_CORES)))
    kernel.last_result = res
    return np.concatenate(
        [res.results[c]["out_shard"] for c in range(N_CORES)], axis=0)


kernel.last_result = None



# revision 11
# speedup vs baseline: 1.0518x; 1.0518x over previous
"""Trainium2 Bass kernel for BlockChunkedActivityRoutedNet.

Reference computation (B=4096, IN_F=4096, 8 chunks of 512, top-2 by mean|x|,
chunk-expert Linears 512->512, concat -> final Linear 1024->4096):

    xr = x.reshape(B, 8, 512)
    activities = mean(|xr|, axis=(0, 2))            # over the WHOLE batch
    i0, i1 = top2(activities)                        # descending
    h = concat(xr[:, i0] @ Wc[i0] + bc[i0], xr[:, i1] @ Wc[i1] + bc[i1])
    out = h @ W_final + b_final

Distribution: data-parallel over the batch across 8 NeuronCores (512 rows
each). Cross-core activity exchange is done with 8 XOR-relative
remote_dma_broadcast rounds (peer SBUF writes + semaphores) instead of a
runtime AllReduce collective: the collective's barrier + mesh machinery cost
40-70us on the critical path, while the remote-DMA exchange costs a few us.
A compile-time prelude AllGather (bir_kernel_barrier_wait) guarantees every
core has entered the kernel (and cleared its semaphores in the preamble)
before any peer's remote write can land.

Host-side prep inside kernel(): casts to bf16 and packs each core's x shard
and the chunk weights as [1024, 2048] tables

    xg[c*128 + p, kt*512 + b] = x.T[c*512 + kt*128 + p, b]
    wg[c*128 + p, kt*512 + d] = W_chunks[c, kt*128 + p, d]

so that (a) chunk c's activity tile is the static row-slice [c*128:(c+1)*128]
with 4KB contiguous lines per partition, and (b) the post-routing gather of a
selected chunk is ONE indirect row-gather [128, 2048] (row = sel*128 + p)
with 4KB lines, instead of 8 1KB-row gathers.

Per core:
  activities: 8 chunk tiles [128, 2048] bf16, |.|-reduce over free dim
              (DVE half / ScalarE Abs+accum half) -> actcol [128, 8] f32.
  exchange:   8 remote_dma_broadcast rounds; round r sends actcol to core
              (me XOR r)'s recv[:, r*8:(r+1)*8]. After recv_sem==16, a
              pairwise tree sum over slots + ones-matmul partition reduce
              gives the global [1, 8] activity sums on every core.
  routing:    top2 via vector.max/max_index; indices broadcast to 128
              partitions via K=1 matmul; per-partition row offsets by iota
              math; chunk biases selected on-chip (PE-transpose + one-hot).
  gather:     2+2 indirect row-gathers (x and W tables, one per selected
              chunk).
  L1:         hT[s][d] = matmul over 4 k-tiles; bias added during PSUM
              eviction -> bf16.
  L2:         out = matmul over 8 k-tiles vs W_final ([128, 4096] bf16
              tiles); b_final added by DVE during PSUM eviction; bf16 out
              DMA (host upcasts to f32).
"""

import numpy as np
import ml_dtypes

import concourse.bass as bass
import concourse.bacc as bacc
import concourse.mybir as mybir
from concourse.tile import TileContext
from concourse.bass_utils import run_bass_kernel_spmd
from concourse.masks import make_identity

dt = mybir.dt
P = 128

NUM_CHUNKS = 8
TOP_K = 2
IN_F = 4096
HID_F = 4096
OUT_F = 4096
B = 4096
CIN = IN_F // NUM_CHUNKS      # 512
COUT = HID_F // NUM_CHUNKS    # 512
N_CORES = 8
BS = B // N_CORES             # 512 rows per core

BT = BS // P                  # 4 batch tiles per core
KT = CIN // P                 # 4 k-tiles per selected chunk
DT_ = COUT // P               # 4 d-tiles per selected chunk
KF = TOP_K * DT_              # 8 k-tiles for the final matmul
OT = OUT_F // 512             # 8 output column tiles of 512
GW = KT * BS                  # 2048 packed-table row width

_cache = {}


def _build():
    nc = bacc.Bacc(num_devices=N_CORES, name="chunk_routed_net",
                   num_swdge_queues=4)

    xg = nc.dram_tensor("xg_shard", [NUM_CHUNKS * P, GW], dt.bfloat16,
                        kind="ExternalInput")
    wg = nc.dram_tensor("wg_chunks", [NUM_CHUNKS * P, GW], dt.bfloat16,
                        kind="ExternalInput")
    bc_t = nc.dram_tensor("b_chunks", [NUM_CHUNKS, COUT], dt.float32,
                          kind="ExternalInput")
    Wf = nc.dram_tensor("W_final", [COUT * TOP_K, OUT_F], dt.bfloat16,
                        kind="ExternalInput")
    bf = nc.dram_tensor("b_final", [1, OUT_F], dt.float32, kind="ExternalInput")
    out = nc.dram_tensor("out_shard", [BS, OUT_F], dt.bfloat16,
                         kind="ExternalOutput")

    # cross-core exchange semaphores (cleared by the per-exec preamble; the
    # entry AllGather below orders every peer's broadcast after every core's
    # preamble)
    recv_sem = nc.alloc_semaphore("act_recv_sem")
    send_sem = nc.alloc_semaphore("act_send_sem")

    ag_in = nc.dram_tensor("entry_ag_in", [1, 1], dt.float32)
    ag_out = nc.dram_tensor("entry_ag_out", [N_CORES, 1], dt.float32)

    with TileContext(nc) as tc:
        with tc.tile_pool(name="consts", bufs=1) as consts, \
             tc.tile_pool(name="route", bufs=1) as route, \
             tc.tile_pool(name="xl", bufs=1) as xl_pool, \
             tc.tile_pool(name="gath", bufs=1) as gath, \
             tc.tile_pool(name="hts", bufs=1) as hts, \
             tc.tile_pool(name="bfinp", bufs=1) as bfinp, \
             tc.tile_pool(name="wfs", bufs=8) as wfs, \
             tc.tile_pool(name="outs", bufs=4) as outs:

            # entry barrier: an AllGather triggered at kernel start; the
            # gpsimd dma_start of its output completes only after every core
            # has entered (and cleared sems in its preamble), and everything
            # later on the gpsimd queue is ordered after that wait
            nc.gpsimd.collective_compute(
                "AllGather", mybir.AluOpType.bypass,
                replica_groups=[list(range(N_CORES))],
                ins=[ag_in.ap()], outs=[ag_out.ap()])

            # ---------------- constants ----------------
            ones_col = consts.tile([P, 1], dt.float32)     # partition reduce
            nc.vector.memset(ones_col[:], 1.0)
            ones_k1 = consts.tile([1, P], dt.float32)      # K=1 bcast matmul
            nc.vector.memset(ones_k1[:], 1.0)
            ones_k1h = consts.tile([1, P], dt.bfloat16)    # K=1 bf16 bcast
            nc.vector.memset(ones_k1h[:], 1.0)
            ident = consts.tile([P, P], dt.float32)
            make_identity(nc, ident)
            # C_R[p, 0] = p                (gather row offset within a chunk)
            C_R = consts.tile([P, 2], dt.int32)
            nc.gpsimd.iota(C_R[:], pattern=[[0, 2]], base=0,
                           channel_multiplier=1)
            C_Rf = consts.tile([P, 2], dt.float32)
            nc.vector.tensor_copy(C_Rf[:], C_R[:])
            # C8[p, c] = c                (chunk-id iota along free dim)
            C8 = consts.tile([P, NUM_CHUNKS], dt.int32)
            nc.gpsimd.iota(C8[:], pattern=[[1, NUM_CHUNKS]], base=0,
                           channel_multiplier=0)
            C8f = consts.tile([P, NUM_CHUNKS], dt.float32)
            nc.vector.tensor_copy(C8f[:], C8[:])

            with tc.tile_pool(name="ps_early", bufs=1, space="PSUM") as ps_early:
                # ------------ activities from packed x (4KB lines) ---------
                actcol = route.tile([P, NUM_CHUNKS], dt.float32)
                scr = route.tile([P, GW], dt.bfloat16)  # ACT throwaway
                xls = []
                H = GW // 2
                for c in range(NUM_CHUNKS):
                    xlt = xl_pool.tile([P, GW], dt.bfloat16, tag=f"xl{c}",
                                       name=f"xl{c}")
                    nc.sync.dma_start(xlt[:, 0:H],
                                      xg[c * P:(c + 1) * P, 0:H])
                    nc.sync.dma_start(xlt[:, H:GW],
                                      xg[c * P:(c + 1) * P, H:GW])
                    xls.append(xlt)
                for c in range(NUM_CHUNKS):
                    if c % 2 == 0:
                        nc.vector.tensor_reduce(
                            actcol[:, c:c + 1], xls[c][:],
                            axis=mybir.AxisListType.X, op=mybir.AluOpType.add,
                            apply_absolute_value=True)
                    else:
                        nc.scalar.activation(
                            scr[:], xls[c][:],
                            mybir.ActivationFunctionType.Abs,
                            accum_out=actcol[:, c:c + 1])

                # ------------ cross-core exchange (remote DMA) ------------
                # round r sends my actcol to core (me XOR r)'s slot r; the
                # self-round r=0 is skipped (own partial summed from actcol).
                # Each arriving round bumps recv_sem by 2 -> 14 total.
                recv = route.tile([P, N_CORES * NUM_CHUNKS], dt.float32,
                                  name="act_recv")
                entry_flag = route.tile([1, 1], dt.float32, name="entry_flag")
                nc.gpsimd.dma_start(entry_flag[:], ag_out[0:1, 0:1])
                for r in range(1, N_CORES):
                    rdests = [None] * N_CORES
                    rdests[r] = (0, r)
                    nc.gpsimd.remote_dma_broadcast(
                        recv[:, r * NUM_CHUNKS:(r + 1) * NUM_CHUNKS],
                        actcol[:],
                        remote_sem=recv_sem, local_sem=send_sem,
                        rdests=rdests)
                nc.gpsimd.trigger_dma(count=None)

                # ---- work that fills the exchange wait ----
                # W_final prefetch: 8 x [128, 4096] bf16 (8KB lines), scalar q
                wf_t = []
                for kf in range(KF):
                    w = wfs.tile([P, OUT_F], dt.bfloat16, tag="wf",
                                 name=f"wf{kf}")
                    nc.scalar.dma_start(w[:], Wf[kf * P:(kf + 1) * P, :])
                    wf_t.append(w)
                # b_final broadcast [128, 4096]
                bfin = bfinp.tile([1, OUT_F], dt.float32)
                nc.scalar.dma_start(bfin[:], bf[:])
                bfin_h = bfinp.tile([1, OUT_F], dt.bfloat16)
                nc.vector.tensor_copy(bfin_h[:], bfin[:])
                bfin_bc = bfinp.tile([P, OUT_F], dt.float32)
                for o in range(OT):
                    sl = slice(o * 512, (o + 1) * 512)
                    ps_b = ps_early.tile([P, 512], dt.float32, tag="psb")
                    nc.tensor.matmul(ps_b[:], ones_k1h[:], bfin_h[:, sl],
                                     start=True, stop=True)
                    nc.vector.tensor_copy(bfin_bc[:, sl], ps_b[:])
                # b_chunks transpose
                b_sb = route.tile([NUM_CHUNKS, COUT], dt.float32)
                nc.scalar.dma_start(b_sb[:], bc_t[:])
                bT = route.tile([P, DT_ * NUM_CHUNKS], dt.float32)
                for d in range(DT_):
                    ps_t = ps_early.tile([P, NUM_CHUNKS], dt.float32, tag="pst")
                    nc.tensor.transpose(
                        ps_t[:], b_sb[:, d * P:(d + 1) * P],
                        ident[0:NUM_CHUNKS, 0:NUM_CHUNKS])
                    nc.scalar.copy(bT[:, d * NUM_CHUNKS:(d + 1) * NUM_CHUNKS],
                                   ps_t[:])

                # ------------ global activity sums ------------
                # acc = actcol + slot1 + ... + slot7. The first add carries a
                # hardware-only recv_sem>=14 wait injected post-scheduling
                # (the Tile sim can't model remote sem delivery).
                acts8 = route.tile([P, NUM_CHUNKS], dt.float32)
                first_add = nc.vector.tensor_tensor(
                    out=acts8[:], in0=actcol[:],
                    in1=recv[:, NUM_CHUNKS:2 * NUM_CHUNKS],
                    op=mybir.AluOpType.add)
                nc._act_recv_wait_fixup = (first_add.ins, recv_sem)
                for r in range(2, N_CORES):
                    nc.vector.tensor_tensor(
                        out=acts8[:], in0=acts8[:],
                        in1=recv[:, r * NUM_CHUNKS:(r + 1) * NUM_CHUNKS],
                        op=mybir.AluOpType.add)
                act_ps = ps_early.tile([1, NUM_CHUNKS], dt.float32, tag="psa")
                nc.tensor.matmul(act_ps[:], ones_col[:], acts8[:],
                                 start=True, stop=True)
                act_g = route.tile([1, NUM_CHUNKS], dt.float32)
                nc.scalar.copy(act_g[:], act_ps[:])

                # ------------ top-2 ------------
                maxv = route.tile([1, NUM_CHUNKS], dt.float32)
                maxi = route.tile([1, NUM_CHUNKS], dt.uint32)
                nc.vector.max(maxv[:], act_g[:])
                nc.vector.max_index(maxi[:], maxv[:], act_g[:])
                maxi_f = route.tile([1, NUM_CHUNKS], dt.float32)
                nc.vector.tensor_copy(maxi_f[:], maxi[:])

                # bcast[p, j] = idx[j] on every partition (K=1 matmul)
                bc_ps = ps_early.tile([P, NUM_CHUNKS], dt.float32, tag="psc")
                nc.tensor.matmul(bc_ps[:], ones_k1[:], maxi_f[:],
                                 start=True, stop=True)
                bcast = route.tile([P, NUM_CHUNKS], dt.float32)
                nc.vector.tensor_copy(bcast[:], bc_ps[:])

            # gather offsets: offR[p, s] = sel_s*128 + p
            bc128 = route.tile([P, TOP_K], dt.float32)
            nc.vector.tensor_scalar_mul(bc128[:], bcast[:, 0:TOP_K], 128.0)
            offR_f = route.tile([P, TOP_K], dt.float32)
            nc.vector.tensor_tensor(
                out=offR_f[:], in0=C_Rf[:, 0:TOP_K], in1=bc128[:],
                op=mybir.AluOpType.add)
            offR = route.tile([P, TOP_K], dt.int32)
            nc.vector.tensor_copy(offR[:], offR_f[:])

            # ------------ gathers: one row-gather per (tensor, slot) -------
            xgt = [gath.tile([P, GW], dt.bfloat16, tag=f"xg{s}", name=f"xg{s}")
                   for s in range(TOP_K)]
            wgt = [gath.tile([P, GW], dt.bfloat16, tag=f"wg{s}", name=f"wg{s}")
                   for s in range(TOP_K)]
            for s in range(TOP_K):
                nc.gpsimd.indirect_dma_start(
                    out=xgt[s][:], out_offset=None,
                    in_=xg[:],
                    in_offset=bass.IndirectOffsetOnAxis(
                        ap=offR[:, s:s + 1], axis=0))
                nc.gpsimd.indirect_dma_start(
                    out=wgt[s][:], out_offset=None,
                    in_=wg[:],
                    in_offset=bass.IndirectOffsetOnAxis(
                        ap=offR[:, s:s + 1], axis=0))

            # chunk-bias select: bias[s][d][p] = bT[p, d*8 + sel_s]
            onehot = route.tile([P, TOP_K * NUM_CHUNKS], dt.float32)
            for s in range(TOP_K):
                nc.vector.tensor_scalar(
                    onehot[:, s * NUM_CHUNKS:(s + 1) * NUM_CHUNKS], C8f[:],
                    bcast[:, s:s + 1], scalar2=None,
                    op0=mybir.AluOpType.is_equal)
            bsel = [[route.tile([P, 1], dt.float32, tag=f"bs{s}_{d}",
                                name=f"bs{s}_{d}")
                     for d in range(DT_)] for s in range(TOP_K)]
            btmp = route.tile([P, NUM_CHUNKS], dt.float32)
            for s in range(TOP_K):
                for d in range(DT_):
                    nc.vector.tensor_tensor(
                        out=btmp[:], in0=bT[:, d * NUM_CHUNKS:(d + 1) * NUM_CHUNKS],
                        in1=onehot[:, s * NUM_CHUNKS:(s + 1) * NUM_CHUNKS],
                        op=mybir.AluOpType.mult)
                    nc.vector.tensor_reduce(
                        bsel[s][d][:], btmp[:], axis=mybir.AxisListType.X,
                        op=mybir.AluOpType.add)

            with tc.tile_pool(name="ps_h", bufs=2, space="PSUM") as ps_h, \
                 tc.tile_pool(name="ps_o", bufs=6, space="PSUM") as ps_o:
                # ------------ L1: hT[s][d] = (x_sel @ Wc[sel]).T + b -------
                hT = [[hts.tile([P, BS], dt.bfloat16, tag=f"ht{s}_{d}",
                                name=f"ht{s}_{d}")
                       for d in range(DT_)] for s in range(TOP_K)]

                def l1_chunk(s):
                    for d in range(DT_):
                        ph = ps_h.tile([P, BS], dt.float32, tag="ph",
                                       name=f"ph{s}_{d}")
                        for kt in range(KT):
                            base = kt * 512 + d * P
                            nc.tensor.matmul(
                                ph[:], wgt[s][:, base:base + P],
                                xgt[s][:, kt * 512:(kt + 1) * 512],
                                start=(kt == 0), stop=(kt == KT - 1))
                        nc.scalar.activation(
                            hT[s][d][:], ph[:],
                            mybir.ActivationFunctionType.Identity,
                            bias=bsel[s][d][:, 0:1])

                l1_chunk(0)

                # pre-start six psum groups (all of o=0 plus o=1 bt=0,1)
                # on slot-0 hT while slot-1 gathers/L1 are still in flight
                PRE = [(0, 0), (0, 1), (0, 2), (0, 3), (1, 0), (1, 1)]
                pre = {}
                for (po_, bt) in PRE:
                    po = ps_o.tile([P, 512], dt.float32, tag="po",
                                   name=f"po_pre{po_}_{bt}")
                    osl = slice(po_ * 512, (po_ + 1) * 512)
                    for kf in range(DT_):
                        nc.tensor.matmul(
                            po[:], hT[0][kf][:, bt * P:(bt + 1) * P],
                            wf_t[kf][:, osl],
                            start=(kf == 0), stop=False)
                    pre[(po_, bt)] = po

                l1_chunk(1)

                # ------------ L2: out = h @ W_final + b_final --------------
                for o in range(OT):
                    osl = slice(o * 512, (o + 1) * 512)
                    for bt in range(BT):
                        if (o, bt) in pre:
                            po = pre[(o, bt)]
                            kfs = range(DT_, KF)
                        else:
                            po = ps_o.tile([P, 512], dt.float32, tag="po",
                                           name=f"po{o}_{bt}")
                            kfs = range(KF)
                        for kf in kfs:
                            s, d = divmod(kf, DT_)
                            nc.tensor.matmul(
                                po[:], hT[s][d][:, bt * P:(bt + 1) * P],
                                wf_t[kf][:, osl],
                                start=(kf == 0), stop=(kf == KF - 1))
                        ot_sb = outs.tile([P, 512], dt.bfloat16, tag="ot",
                                          name=f"ot{o}_{bt}")
                        nc.vector.tensor_tensor(
                            out=ot_sb[:], in0=po[:], in1=bfin_bc[:, osl],
                            op=mybir.AluOpType.add)
                        nc.sync.dma_start(
                            out[bt * P:(bt + 1) * P, osl], ot_sb[:])

    # hardware-only gate: the slot-sum must not start until all 7 peer
    # broadcasts have landed (recv_sem >= 14). Injected after Tile
    # scheduling so the single-core scheduling sim (which cannot model
    # remote sem delivery) doesn't deadlock on it.
    import bass_rust
    ins, sem = nc._act_recv_wait_fixup
    si = ins.sync_info
    new_wait = bass_rust.SyncWait(
        sync_type="semaphore", id=sem.num, ant_name=sem.name,
        wait_mode="sem-ge-imm", wait_value=2 * (N_CORES - 1), wait_reg=None)
    ins.sync_info = bass_rust.SyncInfo(
        on_wait=list(si.on_wait) + [new_wait],
        on_update=list(si.on_update))
    assert "act_recv_sem" in str(ins.sync_info)

    nc.compile()
    return nc


def _pack_table(a):
    # [8, 512, N] -> [1024, 4*N] with row (c*128+p) = a[c, kt*128+p, :] for
    # kt = 0..3 laid side by side
    n = a.shape[-1]
    return np.ascontiguousarray(
        a.reshape(NUM_CHUNKS, KT, P, n).transpose(0, 2, 1, 3)
        .reshape(NUM_CHUNKS * P, KT * n))


def kernel(x, W_chunks, b_chunks, W_final, b_final):
    bf16 = ml_dtypes.bfloat16
    x = np.asarray(x, dtype=np.float32).astype(bf16)
    W_chunks = np.asarray(W_chunks, dtype=np.float32).astype(bf16)
    W_final = np.asarray(W_final, dtype=np.float32).astype(bf16)
    b_chunks = np.ascontiguousarray(np.asarray(b_chunks, dtype=np.float32))
    b_final = np.ascontiguousarray(
        np.asarray(b_final, dtype=np.float32).reshape(1, OUT_F))

    wg = _pack_table(W_chunks)

    if "nc" not in _cache:
        _cache["nc"] = _build()
    nc = _cache["nc"]

    in_maps = []
    for c in range(N_CORES):
        shard = x[c * BS:(c + 1) * BS]              # [512, 4096]
        xt = shard.T.reshape(NUM_CHUNKS, CIN, BS)   # [8, 512, 512]
        in_maps.append({
            "xg_shard": _pack_table(xt),
            "wg_chunks": wg,
            "b_chunks": b_chunks,
            "W_final": W_final,
            "b_final": b_final,
        })

    res = run_bass_kernel_spmd(nc, in_maps, core_ids=list(range(N_CORES)))
    kernel.last_result = res
    return np.concatenate(
        [res.results[c]["out_shard"].astype(np.float32)
         for c in range(N_CORES)], axis=0)


kernel.last_result = None
